# revision 1
# baseline (speedup 1.0000x reference)
"""CensusLoss Trainium2 kernel.

Census transform loss: grayscale -> 48 shifted binary comparisons (7x7 patch,
reflect pad 3) -> mean |pred_census - target_census|.

Sharding: pure data parallel, batch dim B=8 across 8 NeuronCores (one image
per core). Each core emits exact integer partial sums (in f32); the host
combines them and divides.

Per-core pipeline:
  1. gray = 0.299R + 0.587G + 0.114B (ACT muls -> bf16, DVE adds), written
     column-reflect-padded DIRECTLY into the center rows of the "band" tile
     (row width 520 keeps every bf16 row 4B-aligned => DVE 2x_1P mode).
  2. band layout: partition p holds padded rows 4p..4p+9 flattened
     ([128, 5200]); only the 3+3 halo rows need DMAs (partition-shifted
     SBUF->SBUF affine copies from the neighbors' center rows), plus per-row
     reflect copies at the image edges. bandB = bandA shifted one element
     (keeps odd-column-offset neighbor reads 4B-aligned).
  3. Per offset (di,dj): cmpP = is_gt(center, neighbor), cmpT likewise
     (bf16 2x mode, ~1us per [128,2048] op). Every 6th offset instead
     computes d = center - neighbor on the otherwise-idle GPSIMD engine and
     binarizes on DVE with tensor_scalar(d > 0) in 4x mode (bf16 subtraction
     sign is exact, so results are identical).
     sum(xor) = sum(cmpP) + sum(cmpT) - 2*sum(cmpP*cmpT):
       - sum(cmpP): ACT activation(Copy) with accum_out (idle engine)
       - sum(cmpT): PE ones-matmul accumulated in PSUM
       - sum(cmpP*cmpT): PE gram blocks accumulated in PSUM; only the
         diagonal of the [128,128] result is meaningful.
  4. Host: total = sum(acc48) + sum(sums) - 2*trace(prod), exact integers.

Comparisons run in bf16: f32->bf16 rounding is monotonic, so only near-ties
can flip a comparison; measured effect on the mean is ~2e-6 relative.
"""

import numpy as np

B, C, H, W = 8, 3, 512, 512
N_CORES = 8
PAD = 3
N_OFF = 48
Wp = 520            # padded row width (518 used + 2 spare, even for alignment)
COL0 = 4            # padded col of gray col 0 (even => 4B-aligned in bf16)
RPP = 4             # gray rows per partition (512 / 128)
BAND_ROWS = RPP + 2 * PAD            # 10
BAND_LEN = BAND_ROWS * Wp            # 5200
ROW_TILE = RPP * Wp                  # 2080
FREE = RPP * W                       # 2048

_CACHE = {}


def _offsets():
    # even-dj offsets first: they only need the bandA construction, so the
    # main loop starts while the shifted bandB copies are still in flight
    evens, odds = [], []
    for di in range(-PAD, PAD + 1):
        for dj in range(-PAD, PAD + 1):
            if di == 0 and dj == 0:
                continue
            (evens if dj % 2 == 0 else odds).append((di, dj))
    return evens + odds


def _build_bass(n_off=N_OFF, repeat=1):
    from concourse import bacc, mybir
    from concourse.ap import AP
    from concourse.tile import TileContext
    from concourse.alu_op_type import AluOpType as op

    dt = mybir.dt
    # Bacc (not raw Bass): its compile() pass splits multi-sem waits into
    # event-semaphore NOPs — TRN2 instructions allow at most one wait each.
    nc = bacc.Bacc("TRN2", debug=False)

    pred = nc.dram_tensor("pred", [C, H, W], dt.float32, kind="ExternalInput")
    target = nc.dram_tensor("target", [C, H, W], dt.float32, kind="ExternalInput")
    acc48_out = nc.dram_tensor("acc48_out", [128, max(n_off, 1)], dt.float32,
                               kind="ExternalOutput")
    sums_out = nc.dram_tensor("sums_out", [1, 512], dt.float32,
                              kind="ExternalOutput")
    prod_out = nc.dram_tensor("prod_out", [128, 128], dt.float32,
                              kind="ExternalOutput")

    def band_view(t, r0, c0):
        # [128, RPP rows, W cols] view of a band tile at row r0, col c0
        return t.rearrange("p (r w) -> p r w", w=Wp)[
            :, r0:r0 + RPP, c0:c0 + W]

    with TileContext(nc) as tc:
      with tc.tile_pool(name="sbuf", bufs=1) as pool:
        for _rep in range(repeat):
            bands = {}
            for nm in ("p", "t"):
                for ab in ("A", "B"):
                    bands[nm + ab] = pool.tile(
                        [128, BAND_LEN], dt.bfloat16,
                        name=f"band_{nm}{ab}", tag=f"band_{nm}{ab}",
                    )

            # channel loads interleaved across the two HWDGE queues (SP +
            # ACT-seq) with pred's channels at the FRONT of both queues:
            # pred finishes first so its gray/band build overlaps target's
            # remaining input transfers
            chs = {}
            load_order = [("p", 0, nc.sync), ("p", 1, nc.scalar),
                          ("p", 2, nc.sync), ("t", 0, nc.scalar),
                          ("t", 1, nc.sync), ("t", 2, nc.scalar)]
            for nm, c, q in load_order:
                src = pred if nm == "p" else target
                cht = pool.tile([128, FREE], dt.float32,
                                name=f"ch_{nm}{c}", tag=f"ch_{nm}{c}", bufs=1)
                q.dma_start(
                    out=cht,
                    in_=src.ap()[c].rearrange("(p r) w -> p (r w)", p=128),
                )
                chs[(nm, c)] = cht

            for nm, src in (("p", pred), ("t", target)):
                qeng = nc.sync if nm == "p" else nc.scalar
                ch = [chs[(nm, c)] for c in range(3)]
                g1 = pool.tile([128, FREE], dt.bfloat16, name=f"g1_{nm}",
                               tag="g1", bufs=1)
                nc.scalar.mul(g1, ch[0], 0.299)
                gb = pool.tile([128, FREE], dt.bfloat16, name=f"gb_{nm}",
                               tag="gb", bufs=1)
                nc.scalar.mul(gb, ch[1], 0.587)
                gc = pool.tile([128, FREE], dt.bfloat16, name=f"gc_{nm}",
                               tag="gc", bufs=1)
                nc.scalar.mul(gc, ch[2], 0.114)
                g2 = pool.tile([128, FREE], dt.bfloat16, name=f"g2_{nm}",
                               tag="g2", bufs=1)
                nc.vector.tensor_add(g2, g1, gb)
                g3 = pool.tile([128, FREE], dt.bfloat16, name=f"g3_{nm}",
                               tag="g3", bufs=1)
                nc.vector.tensor_add(g3, g2, gc)

                g3v = g3.rearrange("p (r w) -> p r w", w=W)
                # gray rows are written straight into the band tile's center
                # slots (rows 3..6): bandA then only needs the halo DMAs
                bA = bands[nm + "A"]
                padv = bA.rearrange("p (r w) -> p r w", w=Wp)[:, PAD:PAD + RPP, :]
                # zero the 2 spare cols (0 and 519) so halo DMAs carry
                # defined bytes
                nc.vector.memset(
                    AP(bA.tensor, bA.offset + PAD * Wp,
                       [[BAND_LEN, 128], [Wp, RPP], [Wp - 1, 2]]),
                    0.0)
                # center cols: gray col w -> padded col w+COL0
                nc.vector.tensor_copy(out=padv[:, :, COL0:COL0 + W], in_=g3v)
                # reflect cols: padded col COL0-t = gray col t (t=1..3)
                nc.vector.tensor_copy(out=padv[:, :, 1:4], in_=g3v[:, :, 3:0:-1])
                # padded col COL0+W-1+t = gray col W-1-t
                nc.vector.tensor_copy(out=padv[:, :, 516:519],
                                      in_=g3v[:, :, 510:507:-1])

            # ---- halo construction, all SBUF->SBUF within the band ----
            # center slot s (gray row 4p+s) lives at band offset (3+s)*Wp
            for nm in ("t", "p"):
                qeng = nc.sync if nm == "p" else nc.scalar
                bA = bands[nm + "A"]
                pstride_b = bA.ap[0][0]
                # top halo: band[p][slots 0..2] <- band[p-1][center slots 1..3]
                qeng.dma_start(
                    out=AP(bA.tensor, bA.offset + 1 * pstride_b,
                           [[pstride_b, 127], [1, 3 * Wp]]),
                    in_=AP(bA.tensor, bA.offset + 4 * Wp,
                           [[pstride_b, 127], [1, 3 * Wp]]))
                # bottom halo: band[p][slots 7..9] <- band[p+1][center 0..2]
                qeng.dma_start(
                    out=AP(bA.tensor, bA.offset + 7 * Wp,
                           [[pstride_b, 127], [1, 3 * Wp]]),
                    in_=AP(bA.tensor, bA.offset + 1 * pstride_b + 3 * Wp,
                           [[pstride_b, 127], [1, 3 * Wp]]))
                # reflect edges: partition 0 top = gray rows 3,2,1 (center
                # slots 3,2,1); partition 127 bottom = gray rows 510,509,508
                # (center slots 2,1,0)
                for s_band, slot in ((0, 3), (1, 2), (2, 1)):
                    qeng.dma_start(
                        out=AP(bA.tensor, bA.offset + s_band * Wp,
                               [[pstride_b, 1], [1, Wp]]),
                        in_=AP(bA.tensor, bA.offset + (PAD + slot) * Wp,
                               [[pstride_b, 1], [1, Wp]]))
                for s_band, slot in ((7, 2), (8, 1), (9, 0)):
                    qeng.dma_start(
                        out=AP(bA.tensor,
                               bA.offset + 127 * pstride_b + s_band * Wp,
                               [[pstride_b, 1], [1, Wp]]),
                        in_=AP(bA.tensor,
                               bA.offset + 127 * pstride_b + (PAD + slot) * Wp,
                               [[pstride_b, 1], [1, Wp]]))
            # bandB = bandA shifted left one element (last element unused and
            # never read by any compute view)
            for nm in ("p", "t"):
                qeng = nc.sync if nm == "p" else nc.scalar
                bA, bB = bands[nm + "A"], bands[nm + "B"]
                qeng.dma_start(out=bB[:, 0:BAND_LEN - 1],
                               in_=bA[:, 1:BAND_LEN])

            # ---- main loop ----
            centers = {nm: band_view(bands[nm + "A"], PAD, COL0)
                       for nm in ("p", "t")}
            acc48 = pool.tile([128, max(n_off, 1)], dt.float32,
                              name="acc48", tag="acc48")
            nc.vector.memset(acc48, 0.0)
            ones = pool.tile([128, 1], dt.bfloat16, name="ones", tag="ones")
            nc.vector.memset(ones, 1.0)
            with tc.tile_pool(name="psum", bufs=1, space="PSUM") as ppool:
                prod = ppool.tile([128, 128], dt.float32, name="prod")
                sums = ppool.tile([1, 512], dt.float32, name="sums")
                offs = _offsets()[:n_off]
                # every 8th offset's cmpP sum goes to PE instead of ACT
                pe_sum_idx = {i for i in range(len(offs)) if i % 8 == 7}
                # a subset of offsets computes d = center - neighbor on the
                # (otherwise idle) GPSIMD engine, then binarizes on DVE with
                # tensor_scalar(is_gt, 0) in 4x mode — bf16 subtraction sign
                # is exact, so results are identical to a direct is_gt
                gp_n = int(_CACHE.get("gp_n", 8))
                gp_idx = {i for i in range(len(offs)) if i % 6 == 5}
                gp_idx = set(sorted(gp_idx)[:gp_n])
                for i, (di, dj) in enumerate(offs):
                    cmps = {}
                    for nm in ("p", "t"):
                        if dj % 2 == 0:
                            nb = band_view(bands[nm + "A"], PAD + di, COL0 + dj)
                        else:
                            nb = band_view(bands[nm + "B"], PAD + di,
                                           COL0 + dj - 1)
                        cmp = pool.tile([128, FREE], dt.bfloat16,
                                        name=f"cmp_{nm}_{i}", tag=f"cmp_{nm}",
                                        bufs=8)
                        if i in gp_idx:
                            dsub = pool.tile([128, FREE], dt.bfloat16,
                                             name=f"d_{nm}_{i}", tag=f"d_{nm}",
                                             bufs=2)
                            nc.gpsimd.tensor_tensor(
                                out=dsub.rearrange("p (r w) -> p r w", w=W),
                                in0=centers[nm], in1=nb, op=op.subtract)
                            nc.vector.tensor_scalar(
                                out=cmp, in0=dsub, scalar1=0.0, scalar2=None,
                                op0=op.is_gt)
                        else:
                            nc.vector.tensor_tensor(
                                out=cmp.rearrange("p (r w) -> p r w", w=W),
                                in0=centers[nm], in1=nb, op=op.is_gt)
                        cmps[nm] = cmp
                    if i in pe_sum_idx:
                        for c in range(FREE // 512):
                            nc.tensor.matmul(
                                sums[0:1, :], ones[:, 0:1],
                                cmps["p"][:, c * 512:(c + 1) * 512],
                                start=False, stop=False,
                                skip_group_check=True)
                    else:
                        dact = pool.tile([128, FREE], dt.bfloat16,
                                         name=f"dact_{i}", tag="dact", bufs=1)
                        nc.scalar.activation(
                            out=dact, in_=cmps["p"],
                            func=mybir.ActivationFunctionType.Copy,
                            accum_out=acc48[:, i:i + 1])
                    for c in range(FREE // 128):
                        nc.tensor.matmul(
                            prod[:, :],
                            cmps["p"][:, c * 128:(c + 1) * 128],
                            cmps["t"][:, c * 128:(c + 1) * 128],
                            start=(i == 0 and c == 0),
                            stop=(i == len(offs) - 1 and c == FREE // 128 - 1),
                            skip_group_check=True)
                    for c in range(FREE // 512):
                        nc.tensor.matmul(
                            sums[0:1, :], ones[:, 0:1],
                            cmps["t"][:, c * 512:(c + 1) * 512],
                            start=(i == 0 and c == 0),
                            stop=(i == len(offs) - 1 and c == FREE // 512 - 1),
                            skip_group_check=True)

                prod_sb = pool.tile([128, 128], dt.float32, name="prod_sb",
                                    tag="prod_sb")
                sums_sb = pool.tile([1, 512], dt.float32, name="sums_sb",
                                    tag="sums_sb")
                if n_off == 0:
                    nc.vector.memset(prod_sb, 0.0)
                    nc.vector.memset(sums_sb, 0.0)
                else:
                    nc.vector.tensor_copy(out=prod_sb, in_=prod)
                    nc.vector.tensor_copy(out=sums_sb, in_=sums)
                nc.sync.dma_start(out=acc48_out.ap(), in_=acc48)
                nc.sync.dma_start(out=prod_out.ap(), in_=prod_sb)
                nc.sync.dma_start(out=sums_out.ap(), in_=sums_sb)

    nc.finalize()
    return nc


def kernel(pred: np.ndarray, target: np.ndarray) -> np.ndarray:
    from concourse import bass_utils

    if "nc" not in _CACHE:
        _CACHE["nc"] = _build_bass()
    nc = _CACHE["nc"]

    pred = np.ascontiguousarray(pred, dtype=np.float32)
    target = np.ascontiguousarray(target, dtype=np.float32)
    in_maps = [
        {"pred": pred[b], "target": target[b]} for b in range(N_CORES)
    ]
    res = bass_utils.run_bass_kernel_spmd(nc, in_maps,
                                          core_ids=list(range(N_CORES)))
    total = 0.0
    for r in res.results:
        total += float(r["acc48_out"].astype(np.float64).sum())
        total += float(r["sums_out"].astype(np.float64).sum())
        total -= 2.0 * float(np.diag(r["prod_out"]).astype(np.float64).sum())
    mean = total / (B * N_OFF * H * W)
    return np.array(mean, dtype=np.float32)



# revision 9
# speedup vs baseline: 1.9545x; 1.9545x over previous
"""CensusLoss Trainium2 kernel (v2: offset-pairing + PE-side reductions).

Census transform loss: grayscale -> 48 shifted binary comparisons (7x7 patch,
reflect pad 3) -> mean |pred_census - target_census|.

Sharding: pure data parallel, batch dim B=8 across 8 NeuronCores (one image
per core). Host combines per-core integer partial sums and divides.

Math (per core, per offset pair {d, -d}, d = (di,dj) with di>0 or di=0,dj>0):
  Let a = 1{grayP(p) > grayP(p+d)}, b likewise for target, over the 512x512
  interior I. Complement-invariance of XOR plus the near-exact antisymmetry of
  binary comparisons gives
      XOR_d + XOR_{-d} ~= 2 * sum_I (a + b - 2ab),
  with error only from bf16 comparison ties and reflect-boundary strips
  (measured ~1e-5 relative on the real inputs, vs 2e-2 tolerance). So only 24
  comparison maps per image are computed (48 total vs 96 for the direct form).

Per-core pipeline:
  1. Inputs are pre-cast to bf16 on the host (dtype marshalling; halves the
     input DMA bytes). gray = 0.299R + 0.587G + 0.114B: channel scaling on
     ACT, adds on DVE, the final add written column-reflect-padded directly
     into the band tile center rows.
  2. band layout: partition p holds padded rows 4p-3..4p+6 flattened
     ([128, 5200], row width 520). All chosen offsets have di>=0, so only the
     BOTTOM 3 halo rows are needed (partition-shifted SBUF->SBUF copies plus
     per-row reflect copies at partition 127).
  3. 24 pairs x 2 images of is_gt maps ([128, 2048] bf16), split between DVE
     (2x mode) and the otherwise-idle GPSIMD/Pool engine (all of Pool's pred
     maps run before its target maps so it never idles waiting for the
     target band).
  4. Reductions entirely on PE/PSUM, pair groups issued in estimated map
     completion order:
       - sum(a*b): per pair, 16 accumulated [128,128] gram matmuls (only the
         diagonal is meaningful), all pairs into one PSUM bank.
       - sum(a), sum(b): per 128-column map chunk, one matmul with
         rhs=ones[128,1] -> per-column sums at output free size 1 (~1 PE
         row), accumulated across ALL maps into a second PSUM bank.
     A few warmup matmuls during the input DMA phase ramp the PE p-state.
  5. Host: total = 2*(sum(sums) - 2*trace(prod)), exact integers in f32.
"""

import numpy as np

B, C, H, W = 8, 3, 512, 512
N_CORES = 8
PAD = 3
N_OFF = 48
Wp = 520            # padded row width (518 used + 2 spare)
COL0 = 4            # band col of gray col 0
RPP = 4             # gray rows per partition (512 / 128)
BAND_ROWS = RPP + 2 * PAD            # 10
BAND_LEN = BAND_ROWS * Wp            # 5200
FREE = RPP * W                       # 2048

POOL_PAIRS = (4, 8, 12, 16, 20)

_CACHE = {}


def _pairs():
    # the 24 "positive" offsets; their negatives are covered by the pairing
    # identity. di=0 pairs first: they don't depend on the halo DMAs.
    out = [(0, 1), (0, 2), (0, 3)]
    for di in range(1, PAD + 1):
        for dj in range(-PAD, PAD + 1):
            out.append((di, dj))
    assert len(out) == 24
    return out


def _build_bass():
    from concourse import bacc, mybir
    from concourse.ap import AP
    from concourse.tile import TileContext
    from concourse.alu_op_type import AluOpType as op

    dt = mybir.dt
    nc = bacc.Bacc("TRN2", debug=False)

    pred = nc.dram_tensor("pred", [C, H, W], dt.bfloat16, kind="ExternalInput")
    target = nc.dram_tensor("target", [C, H, W], dt.bfloat16,
                            kind="ExternalInput")
    prod_out = nc.dram_tensor("prod_out", [128, 128], dt.float32,
                              kind="ExternalOutput")
    prod2_out = nc.dram_tensor("prod2_out", [128, 128], dt.float32,
                               kind="ExternalOutput")
    sums_out = nc.dram_tensor("sums_out", [128, 1], dt.float32,
                              kind="ExternalOutput")

    pairs = _pairs()
    # pair indices whose two maps run on Pool (GPSIMD); the rest on DVE.
    # Must not include 0,1,2 (the early, halo-free DVE pairs).
    pool_set = set(_CACHE.get("pool_pairs", POOL_PAIRS))
    assert not (pool_set & {0, 1, 2})
    warm_n = int(_CACHE.get("warm_n", 18))

    def band_view(t, s0, c0):
        return t.rearrange("p (r w) -> p r w", w=Wp)[:, s0:s0 + RPP, c0:c0 + W]

    with TileContext(nc) as tc:
      with tc.tile_pool(name="sbuf", bufs=1) as pool:
        bands = {}
        for nm in ("p", "t"):
            bands[nm] = pool.tile([128, BAND_LEN], dt.bfloat16,
                                  name=f"band_{nm}", tag=f"band_{nm}")

        # channel loads: pred first so its gray/band build overlaps target's
        # input transfers; interleave the two HWDGE queues (SP + ACT-seq)
        chs = {}
        load_order = [("p", 0, nc.sync), ("p", 1, nc.scalar),
                      ("p", 2, nc.sync), ("t", 0, nc.scalar),
                      ("t", 1, nc.sync), ("t", 2, nc.scalar)]
        for nm, c, q in load_order:
            cht = pool.tile([128, FREE], dt.bfloat16,
                            name=f"ch_{nm}{c}", tag=f"ch_{nm}{c}", bufs=1)
            src = pred if nm == "p" else target
            q.dma_start(
                out=cht,
                in_=src.ap()[c].rearrange("(p r) w -> p (r w)", p=128),
            )
            chs[(nm, c)] = cht

        ones = pool.tile([128, 1], dt.bfloat16, name="ones", tag="ones")
        nc.vector.memset(ones, 1.0)
        warm = pool.tile([128, 512], dt.bfloat16, name="warm", tag="warm")
        nc.vector.memset(warm, 0.0)

        def gray_mul(nm):
            # ACT channel scalings (ACT is otherwise idle)
            g = {}
            for c, coef in ((0, 0.299), (1, 0.587), (2, 0.114)):
                gt = pool.tile([128, FREE], dt.bfloat16,
                               name=f"g{c}_{nm}", tag=f"g{c}", bufs=1)
                nc.scalar.mul(gt, chs[(nm, c)], coef)
                g[c] = gt
            return g

        def gray_band(nm, g):
            g12 = pool.tile([128, FREE], dt.bfloat16, name=f"g12_{nm}",
                            tag="g12", bufs=1)
            nc.vector.tensor_add(g12, g[0], g[1])
            gf = pool.tile([128, FREE], dt.bfloat16, name=f"gf_{nm}",
                           tag="gf", bufs=1)
            nc.vector.tensor_add(gf, g12, g[2])
            bA = bands[nm]
            padv = bA.rearrange("p (r w) -> p r w", w=Wp)[:, PAD:PAD + RPP, :]
            # zero spare cols 0/519 so the halo DMA carries defined bytes
            nc.vector.memset(
                AP(bA.tensor, bA.offset + PAD * Wp,
                   [[BAND_LEN, 128], [Wp, RPP], [Wp - 1, 2]]),
                0.0)
            gfv = gf.rearrange("p (r w) -> p r w", w=W)
            nc.vector.tensor_copy(out=padv[:, :, COL0:COL0 + W], in_=gfv)
            # reflect cols: band col COL0-t = gray col t (t=1..3)
            nc.vector.tensor_copy(out=padv[:, :, 1:4], in_=gfv[:, :, 3:0:-1])
            nc.vector.tensor_copy(out=padv[:, :, 516:519],
                                  in_=gfv[:, :, 510:507:-1])

        def halos(nm, qeng):
            bA = bands[nm]
            pstride = bA.ap[0][0]
            # bottom halo: band[p][slots 7..9] <- band[p+1][slots 3..5]
            # (rows 4p+4..4p+6); the top halo is never read since all di >= 0
            qeng.dma_start(
                out=AP(bA.tensor, bA.offset + 7 * Wp,
                       [[pstride, 127], [1, 3 * Wp]]),
                in_=AP(bA.tensor, bA.offset + 1 * pstride + 3 * Wp,
                       [[pstride, 127], [1, 3 * Wp]]))
            # partition 127 bottom rows 512..514 = reflect of rows 510..508
            for s_band, slot in ((7, 2), (8, 1), (9, 0)):
                qeng.dma_start(
                    out=AP(bA.tensor,
                           bA.offset + 127 * pstride + s_band * Wp,
                           [[pstride, 1], [1, Wp]]),
                    in_=AP(bA.tensor,
                           bA.offset + 127 * pstride + (PAD + slot) * Wp,
                           [[pstride, 1], [1, Wp]]))

        gp = gray_mul("p")
        gray_band("p", gp)
        halos("p", nc.sync)

        # target ACT muls emitted now so they are not queued behind the
        # pool-route Sign activations on ACT
        gt_ = gray_mul("t")

        with tc.tile_pool(name="psum", bufs=1, space="PSUM") as ppool:
            prod = ppool.tile([128, 128], dt.float32, name="prod")
            prod2 = ppool.tile([128, 128], dt.float32, name="prod2")
            sums = ppool.tile([128, 1], dt.float32, name="sums")
            wps = ppool.tile([1, 512], dt.float32, name="wps")

            # PE p-state warmup during the input-DMA phase
            for _ in range(warm_n):
                nc.tensor.matmul(wps[0:1, :], ones[:, 0:1], warm[:, 0:512],
                                 start=True, stop=True, skip_group_check=True)

            maps = {}

            def make_map(nm, pi, eng):
                di, dj = pairs[pi]
                bA = bands[nm]
                center = band_view(bA, PAD, COL0)
                nbr = band_view(bA, PAD + di, COL0 + dj)
                m = pool.tile([128, FREE], dt.bfloat16,
                              name=f"m_{nm}_{pi}", tag=f"map_{nm}",
                              bufs=21 if nm == "p" else 8)
                if eng == "dve":
                    mv = m.rearrange("p (r w) -> p r w", w=W)
                    nc.vector.tensor_tensor(out=mv, in0=center, in1=nbr,
                                            op=op.is_gt)
                else:
                    # Pool's ALU can't compare, but its subtraction sign is
                    # exact in bf16; ACT Sign turns the diff into a +-1 map
                    dsub = pool.tile([128, FREE], dt.bfloat16,
                                     name=f"d_{nm}_{pi}", tag="dsub", bufs=3)
                    dv = dsub.rearrange("p (r w) -> p r w", w=W)
                    nc.gpsimd.tensor_tensor(out=dv, in0=center, in1=nbr,
                                            op=op.subtract)
                    nc.scalar.sign(out=m, in_=dsub)
                maps[(nm, pi)] = m

            dve_pairs = [i for i in range(24) if i not in pool_set]
            pool_pairs = sorted(pool_set)

            # --- map emission (per-engine program order) ---
            # DVE: early halo-free pred maps; Pool: all pred maps up front
            for pi in (0, 1, 2):
                make_map("p", pi, "dve")
            for pi in pool_pairs:
                make_map("p", pi, "pool")
            gray_band("t", gt_)
            halos("t", nc.scalar)
            # remaining DVE maps: target of the early pairs, then P/T per pair
            for pi in (0, 1, 2):
                make_map("t", pi, "dve")
            for pi in dve_pairs:
                if pi in (0, 1, 2):
                    continue
                make_map("p", pi, "dve")
                make_map("t", pi, "dve")
            for pi in pool_pairs:
                make_map("t", pi, "pool")

            # --- PE reduction groups ---
            # Ordered to match the per-engine map emission order so tile-pool
            # buffer rotation never waits on a PE group scheduled later
            # (DVE pairs stream first; Pool pairs' target maps finish around
            # when the DVE stream ends, so appending them costs nothing).
            n_gram = 0
            n_sum = 0
            N_GRAM_DVE = len(dve_pairs) * 16
            N_SUM = len(dve_pairs) * 32
            for pi in dve_pairs:
                a, b = maps[("p", pi)], maps[("t", pi)]
                for k in range(16):
                    sl = slice(k * 128, (k + 1) * 128)
                    nc.tensor.matmul(prod[:, :], a[:, sl], b[:, sl],
                                     start=(n_gram == 0),
                                     stop=(n_gram == N_GRAM_DVE - 1),
                                     skip_group_check=True)
                    n_gram += 1
                    for mm in (a, b):
                        nc.tensor.matmul(sums[:, 0:1], mm[:, sl],
                                         ones[:, 0:1],
                                         start=(n_sum == 0),
                                         stop=(n_sum == N_SUM - 1),
                                         skip_group_check=True)
                        n_sum += 1
            n_gram2 = 0
            N_GRAM_POOL = len(pool_pairs) * 16
            for pi in pool_pairs:
                a, b = maps[("p", pi)], maps[("t", pi)]
                for k in range(16):
                    sl = slice(k * 128, (k + 1) * 128)
                    nc.tensor.matmul(prod2[:, :], a[:, sl], b[:, sl],
                                     start=(n_gram2 == 0),
                                     stop=(n_gram2 == N_GRAM_POOL - 1),
                                     skip_group_check=True)
                    n_gram2 += 1

            prod_sb = pool.tile([128, 128], dt.float32, name="prod_sb",
                                tag="prod_sb")
            prod2_sb = pool.tile([128, 128], dt.float32, name="prod2_sb",
                                 tag="prod2_sb")
            sums_sb = pool.tile([128, 1], dt.float32, name="sums_sb",
                                tag="sums_sb")
            nc.scalar.copy(out=prod_sb, in_=prod)
            nc.scalar.copy(out=prod2_sb, in_=prod2)
            nc.scalar.copy(out=sums_sb, in_=sums)
            nc.sync.dma_start(out=prod_out.ap(), in_=prod_sb)
            nc.sync.dma_start(out=prod2_out.ap(), in_=prod2_sb)
            nc.sync.dma_start(out=sums_out.ap(), in_=sums_sb)

    nc.finalize()
    return nc


def kernel(pred: np.ndarray, target: np.ndarray) -> np.ndarray:
    import ml_dtypes
    from concourse import bass_utils

    if "nc" not in _CACHE:
        _CACHE["nc"] = _build_bass()
    nc = _CACHE["nc"]

    bf = ml_dtypes.bfloat16
    pred = np.ascontiguousarray(pred, dtype=np.float32).astype(bf)
    target = np.ascontiguousarray(target, dtype=np.float32).astype(bf)
    in_maps = [
        {"pred": pred[b], "target": target[b]} for b in range(N_CORES)
    ]
    res = bass_utils.run_bass_kernel_spmd(nc, in_maps,
                                          core_ids=list(range(N_CORES)))
    n_pool_pairs = len(_CACHE.get("pool_pairs", POOL_PAIRS))
    total = 0.0
    for r in res.results:
        s = float(r["sums_out"].astype(np.float64).sum())
        dg = float(np.diag(r["prod_out"]).astype(np.float64).sum())
        dg2 = float(np.diag(r["prod2_out"]).astype(np.float64).sum())
        total += 2.0 * (s - 2.0 * dg) + (n_pool_pairs * H * W - dg2)
    mean = total / (B * N_OFF * H * W)
    return np.array(mean, dtype=np.float32)
